# revision 17
# baseline (speedup 1.0000x reference)
"""Masked dot-product attention on 8 Trainium2 NeuronCores.

Problem: q,k,v [64, 1024, 64] f32, valid_lens [64] int32.
  scores = q @ k^T / 8, mask keys >= valid_len to -1e6, softmax, @ v.

Strategy (per core: 8 batches, pure data parallelism, no collectives):

  Host prep: batches rank-sorted by valid_len and dealt one per core per
  slot; per-batch key tiles truncated to jt = ceil(valid/128) (masked tail
  tiles contribute exactly zero).  Slot order is [smallest, largest, ...,
  2nd-smallest] so the pipeline fills fast and drains cheap.  All of a
  core's inputs are baked into ONE fp16 DRAM blob [128, W] (one dma_start
  per slot, ~0.4MB each): per slot a q block [128, 512] (rows 0-63 = q^T
  cols 0-511, rows 64-127 = q^T cols 512-1023 -- NO replication), then per
  key tile a k block [128, 128] (k^T tile in rows 0-63, replicated in rows
  64-127) and a vm block [128, 65] = [v*mask | mask] (keys on partitions).
  valid_len==0 batches reproduce the reference's uniform softmax by zeroed
  q + all-ones mask.

  Scores (PE): per pair of key tiles (j0, j1), 2 issue-slots of 2
  concurrent matmuls on disjoint PE row groups (K=64 each, fp16):
  slot A computes j0 x q-cols-lo and j1 x q-cols-hi, slot B the converse,
  so q streams once per key-tile pair with zero operand replication.
  Odd-tail tiles use one slot (lo/hi halves concurrently).

  exp (the softmax bottleneck, ~1.1us/tile of [128,1024]): split between
  TWO engines running in parallel.  ACT tiles: nc.scalar.activation Exp
  (scale=1/8, bias=beta) -> fp16.  DVE tiles: one-pass Schraudolph --
  i16 = f32_scores * (2^10/ln2)/8 + 13900, written through an int16
  bitcast of the fp16 tile; the int16 bit pattern IS ~exp(s/8 + beta) in
  fp16 (max ~3% sawtooth error; errors partly cancel through the shared
  matmul denominator).  beta = ln2*(13900-15316)/1024 matches the two
  paths; it cancels between numerator and denominator.  The +13900 offset
  keeps i16 positive down to s = -75 (9.4 sigma of the N(0,64) scores).
  DVE tiles are placed where they are cheapest in error: only on slots
  with jt >= 3 (small-jt batches concentrate softmax mass per tile), and
  within a slot on the tiles that are masked out for the most batches.
  Each scores pair mixes one ACT and one DVE tile where possible so both
  engines consume every pair concurrently.

  attn@v (PE): po[128q, 65] += exp_tile.T.T @ [vm] per 128-query chunk,
  fp16 weights on the fast-weight-load path, fp32 PSUM.  Chunk groups
  alternate between two PSUM accumulator tiles so two accumulation groups
  overlap.  Emission is drip-fed between the NEXT batch's score/exp pairs
  (2-matmul slivers) so PE alternates between feeding ACT/DVE (scores)
  and draining them.

  Normalize (DVE): per po accumulator one reciprocal [128,4] + one
  broadcast tensor_tensor multiply [128,4,64] -> fp16 osb half; bank 0 is
  normalized and stored (sync HWDGE) while bank 1's groups still run;
  bank 1 stores on the GpSimd SWDGE queue.  Output DRAM is fp16
  [slot, 128, 512] (chunk-major); the host transposes back to [1024, 64]
  f32.
"""

import numpy as np

import concourse.bass as bass
import concourse.bacc as bacc
import concourse.tile as tile
from concourse import mybir
from concourse import bass_utils

B, S, D = 64, 1024, 64
NCORES = 8
NB = B // NCORES  # batch slots per core
P = 128
NJT = S // P  # max key tiles per batch
W = D + 1  # v columns + mask column
F32 = mybir.dt.float32
F16 = mybir.dt.float16
I16 = mybir.dt.int16

# Schraudolph-fp16 exp constants (see module docstring).
A16 = (2.0 ** 10 / np.log(2.0)) / 8.0       # folds the 1/8 score scale
B16 = 13900.0
BETA = float(np.log(2.0) * (B16 - (15 * 1024 - 44)) / 1024.0)  # ~ -0.958

# ACT tile count per jt; the rest go to DVE.  Small-jt slots stay all-ACT.
ACT_OF_JT = {1: 1, 2: 2, 3: 2, 4: 3, 5: 3, 6: 4, 7: 4, 8: 4}

TRACE = False  # set by test harness to capture an NTFF profile
LAST_RESULTS = None  # BassKernelResults stash for the harness

_program_cache = {}


def _slot_layout(jts):
    """Column offsets into the per-core input blob, per slot."""
    qofs, kofs, ofs = [], [], 0
    for jt in jts:
        qofs.append(ofs)
        kofs.append(ofs + 512)
        ofs += 512 + jt * (P + W)
    return qofs, kofs, ofs


def _av_steps(nc, po_pool, osb_pool, rec_pool, in_all, outb, t, jt, kofs, exs,
              last):
    """Yield one emission step at a time: 8 attn@v chunk-groups (2-matmul
    slivers), per-bank normalization + store.  Bank 0's groups run first so
    its normalize + store overlap bank 1's groups.  The caller interleaves
    these steps between the NEXT batch's score/exp pairs."""
    po = [po_pool.tile([P, 4 * W], F32, tag=f"po{h}", name=f"po{h}")
          for h in range(2)]
    osb = osb_pool.tile([P, 8 * D], F16, tag="osb", name="osb")
    osb3 = osb.rearrange("p (c d) -> p c d", d=D)

    def group(qc):
        dst = po[qc // 4]
        col = (qc % 4) * W
        for j in range(jt):
            nc.tensor.matmul(
                dst[:, col:col + W],
                lhsT=exs[j][:, qc * P:(qc + 1) * P],
                rhs=in_all[:, kofs + j * (P + W) + P: kofs + (j + 1) * (P + W)],
                start=(j == 0), stop=(j == jt - 1),
            )
            if j % 2 == 1:
                yield
        yield

    def norm(h):
        po3 = po[h].rearrange("p (c w) -> p c w", w=W)
        recp = rec_pool.tile([P, 4], F32, tag="rec", name="recp")
        nc.vector.reciprocal(out=recp, in_=po3[:, :, D])
        rb = recp.rearrange("p (c o) -> p c o", o=1).broadcast_to([P, 4, D])
        nc.vector.tensor_tensor(
            out=osb3[:, 4 * h:4 * h + 4, :],
            in0=po3[:, :, 0:D], in1=rb,
            op=mybir.AluOpType.mult,
        )
        yield

    for qc in (0, 1, 2, 3):
        yield from group(qc)
    yield from norm(0)
    nc.sync.dma_start(out=outb[t, :, 0:4 * D], in_=osb[:, 0:4 * D])
    yield
    for qc in (4, 5, 6, 7):
        yield from group(qc)
    yield from norm(1)
    # The last slot's final store goes on the sync HWDGE ring so the
    # end-of-program drain waits on one queue, not two.
    eng = nc.sync if last else nc.gpsimd
    eng.dma_start(out=outb[t, :, 4 * D:8 * D], in_=osb[:, 4 * D:8 * D])
    yield


def _build_program(key):
    jts, dve_sets = key
    nc = bacc.Bacc("TRN2", target_bir_lowering=False, debug=False,
                   num_devices=NCORES)
    qofs, kofs, totw = _slot_layout(jts)
    inb = nc.dram_tensor("inb", [P, totw], F16, kind="ExternalInput").ap()
    outb = nc.dram_tensor("out", [NB, P, 8 * D], F16,
                          kind="ExternalOutput").ap()

    with tile.TileContext(nc) as tc:
        with (
            tc.tile_pool(name="singles", bufs=1) as singles,
            tc.tile_pool(name="ex", bufs=2 * NJT) as ex_pool,
            tc.tile_pool(name="osb", bufs=2) as osb_pool,
            tc.tile_pool(name="rec", bufs=4) as rec_pool,
            tc.tile_pool(name="ps_s", bufs=3, space="PSUM") as ps_pool,
            tc.tile_pool(name="ps_o", bufs=1, space="PSUM") as po_pool,
        ):
            in_all = singles.tile([P, totw], F16)
            bias_t = singles.tile([P, 1], F32)
            nc.vector.memset(bias_t, BETA)
            # All input loads upfront on the Sync HWDGE ring: FIFO delivery
            # in slot order stays ahead of compute.
            for t in range(NB):
                w = 512 + jts[t] * (P + W)
                nc.sync.dma_start(out=in_all[:, qofs[t]:qofs[t] + w],
                                  in_=inb[:, qofs[t]:qofs[t] + w])
            # PE warm-up: dummy matmuls fill the first-load DMA wait and
            # start the HAM un-throttle window ~4us early (results land in
            # a recycled PSUM tile, unread).
            warm = singles.tile([D, 512], F16)
            nc.gpsimd.memset(warm, 0.0)
            wps = ps_pool.tile([P, S], F32, tag="ps", name="ps")
            for _ in range(2):
                nc.tensor.matmul(wps[:, 0:512], lhsT=warm[:, 0:P],
                                 rhs=warm, start=True, stop=True,
                                 tile_position=(0, 0))

            def kap(t, j, half):
                c = kofs[t] + j * (P + W)
                return in_all[half * D:(half + 1) * D, c:c + P]

            def qap(t, half):
                return in_all[half * D:(half + 1) * D,
                              qofs[t]:qofs[t] + 512]

            pending = None  # unfinished attn@v/epilogue of previous batch
            drip = 1
            for t in range(NB):
                jt = jts[t]
                dve = dve_sets[t]
                exs = [None] * jt
                # Dense 4-matmul bursts per pair of key tiles keep the PE
                # activity high enough that HAM holds the 2.4 GHz clock.
                acts = [j for j in range(jt) if j not in dve]
                dves = sorted(dve)
                n = min(len(acts), len(dves))
                rest = acts[n:] + dves[n:]
                pairs = [(acts[i], dves[i]) for i in range(n)]
                pairs += [tuple(rest[i:i + 2])
                          for i in range(0, len(rest), 2)]
                for js in pairs:
                    if len(js) == 2:
                        j0, j1 = js
                        ps0 = ps_pool.tile([P, S], F32, tag="ps", name="ps")
                        ps1 = ps_pool.tile([P, S], F32, tag="ps", name="ps")
                        nc.tensor.matmul(ps0[:, 0:512], lhsT=kap(t, j0, 0),
                                         rhs=qap(t, 0), start=True, stop=True,
                                         tile_position=(0, 0))
                        nc.tensor.matmul(ps1[:, 512:1024], lhsT=kap(t, j1, 1),
                                         rhs=qap(t, 1), start=True, stop=True,
                                         tile_position=(D, 0))
                        nc.tensor.matmul(ps0[:, 512:1024], lhsT=kap(t, j0, 1),
                                         rhs=qap(t, 1), start=True, stop=True,
                                         tile_position=(D, 0))
                        nc.tensor.matmul(ps1[:, 0:512], lhsT=kap(t, j1, 0),
                                         rhs=qap(t, 0), start=True, stop=True,
                                         tile_position=(0, 0))
                        tiles = ((j0, ps0), (j1, ps1))
                    else:
                        j0 = js[0]
                        ps0 = ps_pool.tile([P, S], F32, tag="ps", name="ps")
                        nc.tensor.matmul(ps0[:, 0:512], lhsT=kap(t, j0, 0),
                                         rhs=qap(t, 0), start=True, stop=True,
                                         tile_position=(0, 0))
                        nc.tensor.matmul(ps0[:, 512:1024], lhsT=kap(t, j0, 1),
                                         rhs=qap(t, 1), start=True, stop=True,
                                         tile_position=(D, 0))
                        tiles = ((j0, ps0),)
                    for j, ps in tiles:
                        ex = ex_pool.tile([P, S], F16, tag="ex", name="ex")
                        if j in dve:
                            nc.vector.tensor_scalar(
                                out=ex.bitcast(I16), in0=ps,
                                scalar1=float(A16), scalar2=float(B16),
                                op0=mybir.AluOpType.mult,
                                op1=mybir.AluOpType.add)
                        else:
                            nc.scalar.activation(
                                out=ex, in_=ps,
                                func=mybir.ActivationFunctionType.Exp,
                                scale=0.125, bias=bias_t)
                        exs[j] = ex
                        # drain a sliver of the previous batch's attn@v
                        # after each exp (keeps all engines fed)
                        if pending is not None:
                            for _ in range(drip):
                                if next(pending, "done") == "done":
                                    pending = None
                                    break
                if pending is not None:
                    for _ in pending:
                        pass
                pending = _av_steps(nc, po_pool, osb_pool, rec_pool, in_all,
                                    outb, t, jt, kofs[t], exs, t == NB - 1)
                # Pace so scores pairs arrive at least as fast as ACT/DVE
                # consume them (~1.1us/pair): at most 4 drip steps per exp.
                nsteps = 8 * (jt // 2 + 1) + 4
                nxt = jts[t + 1] if t + 1 < NB else jt
                npairs_next = max(1, (nxt + 1) // 2)
                drip = min(4, max(2, -(-nsteps // (2 * npairs_next))))
            for _ in pending:
                pass
    nc.compile()
    return nc


def kernel(q, k, v, valid_lens):
    global LAST_RESULTS
    q = np.array(q, dtype=np.float32, copy=True)
    k = np.asarray(k, dtype=np.float32)
    v = np.asarray(v, dtype=np.float32)
    vl = np.asarray(valid_lens).astype(np.int64)

    # valid_len == 0: reference's softmax over an all-masked row is uniform.
    # Zeroed q gives scores == 0 -> exp == const over all (unmasked) keys.
    valid_eff = np.where(vl <= 0, S, np.minimum(vl, S))
    q[vl <= 0] = 0.0

    mask = (np.arange(S)[None, :] < valid_eff[:, None]).astype(np.float32)
    qT = np.ascontiguousarray(q.transpose(0, 2, 1)).astype(np.float16)
    kT = np.ascontiguousarray(k.transpose(0, 2, 1)).astype(np.float16)
    vm = np.concatenate([v * mask[:, :, None], mask[:, :, None]], axis=2)
    vm = vm.astype(np.float16)

    # Rank-sort batches; slot s takes one batch of rank group [8s, 8s+8)
    # per core.  Schedule order: smallest first (fast fill), then largest
    # down to 2nd-smallest (cheap drain).
    order = np.argsort(-valid_eff, kind="stable")
    asc = order.reshape(NB, NCORES)[::-1]
    jts_asc = [int(np.ceil(valid_eff[asc[s]].max() / P)) for s in range(NB)]
    perm = [0] + list(range(NB - 1, 0, -1))
    assign = asc[perm]                      # [slot t, core c] -> batch
    jts = tuple(jts_asc[p] for p in perm)

    # DVE (Schraudolph) tiles: per slot, the d lowest-mass tiles -- the
    # ones fully masked for the most batches in the slot (ties: highest j).
    dve_sets = []
    for t in range(NB):
        jt = jts[t]
        d = jt - min(ACT_OF_JT[jt], jt)
        nreal = [(int(np.sum(valid_eff[assign[t]] > j * P)), -j)
                 for j in range(jt)]
        dve_sets.append(frozenset(sorted(range(jt),
                                         key=lambda j: nreal[j])[:d]))
    key = (jts, tuple(dve_sets))

    nc = _program_cache.get(key)
    if nc is None:
        nc = _build_program(key)
        _program_cache[key] = nc

    qofs, kofs, totw = _slot_layout(jts)
    in_maps = []
    for c in range(NCORES):
        blob = np.zeros((P, totw), dtype=np.float16)
        for t in range(NB):
            b = assign[t, c]
            jt = jts[t]
            qb = blob[:, qofs[t]:qofs[t] + 512]
            qb[0:D] = qT[b][:, 0:512]
            qb[D:2 * D] = qT[b][:, 512:1024]
            for j in range(jt):
                c0 = kofs[t] + j * (P + W)
                blob[0:D, c0:c0 + P] = kT[b][:, j * P:(j + 1) * P]
                blob[D:2 * D, c0:c0 + P] = kT[b][:, j * P:(j + 1) * P]
                blob[:, c0 + P:c0 + P + W] = vm[b][j * P:(j + 1) * P, :]
        in_maps.append({"inb": blob})
    cores = list(range(NCORES))
    if TRACE:
        # Warm untraced run first: NTFF profiling wrapped around the very
        # first post-compile execute has been seen to wedge the exec unit.
        try:
            bass_utils.run_bass_kernel_spmd(nc, in_maps, core_ids=cores,
                                            trace=False)
        except Exception:
            pass
    try:
        res = bass_utils.run_bass_kernel_spmd(nc, in_maps, core_ids=cores,
                                              trace=TRACE)
    except Exception:
        # One retry: transient NRT exec-unit failures have been observed on
        # the first execute after a fresh in-process NEFF compile.
        res = bass_utils.run_bass_kernel_spmd(nc, in_maps, core_ids=cores,
                                              trace=TRACE)
    LAST_RESULTS = res

    out = np.empty((B, S, D), dtype=np.float32)
    for c in range(NCORES):
        o = res.results[c]["out"]  # [NB, 128, 512] fp16
        for t in range(NB):
            out[assign[t, c]] = (
                o[t].reshape(P, 8, D).transpose(1, 0, 2)
                .reshape(S, D).astype(np.float32)
            )
    return out


# revision 18
# speedup vs baseline: 1.1900x; 1.1900x over previous
"""Masked dot-product attention on 8 Trainium2 NeuronCores.

Problem: q,k,v [64, 1024, 64] f32, valid_lens [64] int32.
  scores = q @ k^T / 8, mask keys >= valid_len to -1e6, softmax, @ v.

Strategy (per core: 8 batches, pure data parallelism, no collectives):

  Host prep: batches rank-sorted by valid_len and dealt one per core per
  slot; per-batch key tiles truncated to jt = ceil(valid/128) (masked tail
  tiles contribute exactly zero).  Slot order is [smallest, largest, ...,
  2nd-smallest] so the pipeline fills fast and drains cheap.  All of a
  core's inputs are baked into ONE fp16 DRAM blob [128, W] (one dma_start
  per slot, ~0.4MB each): per slot a q block [128, 512] (rows 0-63 = q^T
  cols 0-511, rows 64-127 = q^T cols 512-1023 -- NO replication), then per
  key tile a k block [128, 128] (k^T tile in rows 0-63, replicated in rows
  64-127) and a vm block [128, 65] = [v*mask | mask] (keys on partitions).
  valid_len==0 batches reproduce the reference's uniform softmax by zeroed
  q + all-ones mask.

  Scores (PE): per pair of key tiles (j0, j1), 2 issue-slots of 2
  concurrent matmuls on disjoint PE row groups (K=64 each, fp16):
  slot A computes j0 x q-cols-lo and j1 x q-cols-hi, slot B the converse,
  so q streams once per key-tile pair with zero operand replication.
  Odd-tail tiles use one slot (lo/hi halves concurrently).

  exp (the softmax bottleneck, ~1.1us/tile of [128,1024]): split between
  TWO engines running in parallel.  ACT tiles: nc.scalar.activation Exp
  (scale=1/8, bias=beta) -> fp16.  DVE tiles: one-pass Schraudolph --
  i16 = f32_scores * (2^10/ln2)/8 + 13900, written through an int16
  bitcast of the fp16 tile; the int16 bit pattern IS ~exp(s/8 + beta) in
  fp16 (max ~3% sawtooth error; errors partly cancel through the shared
  matmul denominator).  beta = ln2*(13900-15316)/1024 matches the two
  paths; it cancels between numerator and denominator.  The +13900 offset
  keeps i16 positive down to s = -75 (9.4 sigma of the N(0,64) scores).
  DVE tiles are placed where they are cheapest in error: only on slots
  with jt >= 3 (small-jt batches concentrate softmax mass per tile), and
  within a slot on the tiles that are masked out for the most batches.
  Each scores pair mixes one ACT and one DVE tile where possible so both
  engines consume every pair concurrently.

  attn@v (PE): po[128q, 65] += exp_tile.T.T @ [vm] per 128-query chunk,
  fp16 weights on the fast-weight-load path, fp32 PSUM.  Chunk groups
  alternate between two PSUM accumulator tiles so two accumulation groups
  overlap.  Emission is drip-fed between the NEXT batch's score/exp pairs
  (2-matmul slivers) so PE alternates between feeding ACT/DVE (scores)
  and draining them.

  Normalize (DVE): per po accumulator one reciprocal [128,4] + one
  broadcast tensor_tensor multiply [128,4,64] -> fp16 osb half; bank 0 is
  normalized and stored (sync HWDGE) while bank 1's groups still run;
  bank 1 stores on the GpSimd SWDGE queue.  Output DRAM is fp16
  [slot, 128, 512] (chunk-major); the host transposes back to [1024, 64]
  f32.
"""

import numpy as np

import concourse.bass as bass
import concourse.bacc as bacc
import concourse.tile as tile
from concourse import mybir
from concourse import bass_utils

B, S, D = 64, 1024, 64
NCORES = 8
NB = B // NCORES  # batch slots per core
P = 128
NJT = S // P  # max key tiles per batch
W = D + 1  # v columns + mask column
F32 = mybir.dt.float32
F16 = mybir.dt.float16
I16 = mybir.dt.int16

# Schraudolph-fp16 exp constants (see module docstring).
A16 = (2.0 ** 10 / np.log(2.0)) / 8.0       # folds the 1/8 score scale
B16 = 13900.0
BETA = float(np.log(2.0) * (B16 - (15 * 1024 - 44)) / 1024.0)  # ~ -0.958

# ACT tile count per jt; the rest go to DVE.  Small-jt slots stay all-ACT.
ACT_OF_JT = {1: 1, 2: 2, 3: 2, 4: 3, 5: 3, 6: 4, 7: 4, 8: 4}

TRACE = False  # set by test harness to capture an NTFF profile
LAST_RESULTS = None  # BassKernelResults stash for the harness

_program_cache = {}


def _slot_layout(jts):
    """Column offsets into the per-core input blob, per slot."""
    qofs, kofs, ofs = [], [], 0
    for jt in jts:
        qofs.append(ofs)
        kofs.append(ofs + 512)
        ofs += 512 + jt * (P + W)
    return qofs, kofs, ofs


def _av_steps(nc, po_pool, osb_pool, rec_pool, in_all, outb, t, jt, kofs, exs,
              last):
    """Yield one emission step at a time: 8 attn@v chunk-groups (2-matmul
    slivers), per-bank normalization + store.  Bank 0's groups run first so
    its normalize + store overlap bank 1's groups.  The caller interleaves
    these steps between the NEXT batch's score/exp pairs."""
    po = [po_pool.tile([P, 4 * W], F32, tag=f"po{h}", name=f"po{h}")
          for h in range(2)]
    osb = osb_pool.tile([P, 8 * D], F16, tag="osb", name="osb")
    osb3 = osb.rearrange("p (c d) -> p c d", d=D)

    def group(qc):
        dst = po[qc // 4]
        col = (qc % 4) * W
        for j in range(jt):
            nc.tensor.matmul(
                dst[:, col:col + W],
                lhsT=exs[j][:, qc * P:(qc + 1) * P],
                rhs=in_all[:, kofs + j * (P + W) + P: kofs + (j + 1) * (P + W)],
                start=(j == 0), stop=(j == jt - 1),
            )
            if j % 2 == 1:
                yield
        yield

    def norm(h):
        po3 = po[h].rearrange("p (c w) -> p c w", w=W)
        recp = rec_pool.tile([P, 4], F32, tag="rec", name="recp")
        nc.vector.reciprocal(out=recp, in_=po3[:, :, D])
        rb = recp.rearrange("p (c o) -> p c o", o=1).broadcast_to([P, 4, D])
        nc.vector.tensor_tensor(
            out=osb3[:, 4 * h:4 * h + 4, :],
            in0=po3[:, :, 0:D], in1=rb,
            op=mybir.AluOpType.mult,
        )
        yield

    for qc in (0, 1, 2, 3):
        yield from group(qc)
    yield from norm(0)
    nc.sync.dma_start(out=outb[t, :, 0:4 * D], in_=osb[:, 0:4 * D])
    yield
    for qc in (4, 5, 6, 7):
        yield from group(qc)
    yield from norm(1)
    # The last slot's final store goes on the sync HWDGE ring so the
    # end-of-program drain waits on one queue, not two.
    eng = nc.sync if last else nc.gpsimd
    eng.dma_start(out=outb[t, :, 4 * D:8 * D], in_=osb[:, 4 * D:8 * D])
    yield


def _build_program(key):
    jts, dve_sets = key
    nc = bacc.Bacc("TRN2", target_bir_lowering=False, debug=False,
                   num_devices=NCORES)
    qofs, kofs, totw = _slot_layout(jts)
    inb = nc.dram_tensor("inb", [P, totw], F16, kind="ExternalInput").ap()
    outb = nc.dram_tensor("out", [NB, P, 8 * D], F16,
                          kind="ExternalOutput").ap()

    with tile.TileContext(nc) as tc:
        with (
            tc.tile_pool(name="singles", bufs=1) as singles,
            tc.tile_pool(name="ex", bufs=2 * NJT) as ex_pool,
            tc.tile_pool(name="osb", bufs=2) as osb_pool,
            tc.tile_pool(name="rec", bufs=4) as rec_pool,
            tc.tile_pool(name="ps_s", bufs=3, space="PSUM") as ps_pool,
            tc.tile_pool(name="ps_o", bufs=1, space="PSUM") as po_pool,
        ):
            in_all = singles.tile([P, totw], F16)
            bias_t = singles.tile([P, 1], F32)
            nc.vector.memset(bias_t, BETA)
            # All input loads upfront on the Sync HWDGE ring: FIFO delivery
            # in slot order stays ahead of compute.
            for t in range(NB):
                w = 512 + jts[t] * (P + W)
                nc.sync.dma_start(out=in_all[:, qofs[t]:qofs[t] + w],
                                  in_=inb[:, qofs[t]:qofs[t] + w])
            # PE warm-up: dummy matmuls fill the first-load DMA wait and
            # start the HAM un-throttle window ~4us early (results land in
            # a recycled PSUM tile, unread).
            warm = singles.tile([D, 512], F16)
            nc.gpsimd.memset(warm, 0.0)
            wps = ps_pool.tile([P, S], F32, tag="ps", name="ps")
            for _ in range(2):
                nc.tensor.matmul(wps[:, 0:512], lhsT=warm[:, 0:P],
                                 rhs=warm, start=True, stop=True,
                                 tile_position=(0, 0))

            def kap(t, j, half):
                c = kofs[t] + j * (P + W)
                return in_all[half * D:(half + 1) * D, c:c + P]

            def qap(t, half):
                return in_all[half * D:(half + 1) * D,
                              qofs[t]:qofs[t] + 512]

            pending = None  # unfinished attn@v/epilogue of previous batch
            drip = 1
            for t in range(NB):
                jt = jts[t]
                dve = dve_sets[t]
                exs = [None] * jt
                # Dense 4-matmul bursts per pair of key tiles keep the PE
                # activity high enough that HAM holds the 2.4 GHz clock.
                acts = [j for j in range(jt) if j not in dve]
                dves = sorted(dve)
                n = min(len(acts), len(dves))
                rest = acts[n:] + dves[n:]
                pairs = [(acts[i], dves[i]) for i in range(n)]
                pairs += [tuple(rest[i:i + 2])
                          for i in range(0, len(rest), 2)]
                for js in pairs:
                    if len(js) == 2:
                        j0, j1 = js
                        ps0 = ps_pool.tile([P, S], F32, tag="ps", name="ps")
                        ps1 = ps_pool.tile([P, S], F32, tag="ps", name="ps")
                        nc.tensor.matmul(ps0[:, 0:512], lhsT=kap(t, j0, 0),
                                         rhs=qap(t, 0), start=True, stop=True,
                                         tile_position=(0, 0))
                        nc.tensor.matmul(ps1[:, 512:1024], lhsT=kap(t, j1, 1),
                                         rhs=qap(t, 1), start=True, stop=True,
                                         tile_position=(D, 0))
                        nc.tensor.matmul(ps0[:, 512:1024], lhsT=kap(t, j0, 1),
                                         rhs=qap(t, 1), start=True, stop=True,
                                         tile_position=(D, 0))
                        nc.tensor.matmul(ps1[:, 0:512], lhsT=kap(t, j1, 0),
                                         rhs=qap(t, 0), start=True, stop=True,
                                         tile_position=(0, 0))
                        tiles = ((j0, ps0), (j1, ps1))
                    else:
                        j0 = js[0]
                        ps0 = ps_pool.tile([P, S], F32, tag="ps", name="ps")
                        nc.tensor.matmul(ps0[:, 0:512], lhsT=kap(t, j0, 0),
                                         rhs=qap(t, 0), start=True, stop=True,
                                         tile_position=(0, 0))
                        nc.tensor.matmul(ps0[:, 512:1024], lhsT=kap(t, j0, 1),
                                         rhs=qap(t, 1), start=True, stop=True,
                                         tile_position=(D, 0))
                        tiles = ((j0, ps0),)
                    for j, ps in tiles:
                        ex = ex_pool.tile([P, S], F16, tag="ex", name="ex")
                        if j in dve:
                            nc.vector.tensor_scalar(
                                out=ex.bitcast(I16), in0=ps,
                                scalar1=float(A16), scalar2=float(B16),
                                op0=mybir.AluOpType.mult,
                                op1=mybir.AluOpType.add)
                        else:
                            nc.scalar.activation(
                                out=ex, in_=ps,
                                func=mybir.ActivationFunctionType.Exp,
                                scale=0.125, bias=bias_t)
                        exs[j] = ex
                        # drain a sliver of the previous batch's attn@v
                        # after each exp (keeps all engines fed)
                        if pending is not None:
                            for _ in range(drip):
                                if next(pending, "done") == "done":
                                    pending = None
                                    break
                if pending is not None:
                    for _ in pending:
                        pass
                pending = _av_steps(nc, po_pool, osb_pool, rec_pool, in_all,
                                    outb, t, jt, kofs[t], exs, t == NB - 1)
                # Pace so scores pairs arrive at least as fast as ACT/DVE
                # consume them (~1.1us/pair): at most 5 drip steps per exp.
                nsteps = 8 * (jt // 2 + 1) + 4
                nxt = jts[t + 1] if t + 1 < NB else jt
                npairs_next = max(1, (nxt + 1) // 2)
                drip = min(5, max(2, -(-nsteps // (2 * npairs_next))))
            for _ in pending:
                pass
    nc.compile()
    return nc


def kernel(q, k, v, valid_lens):
    global LAST_RESULTS
    q = np.array(q, dtype=np.float32, copy=True)
    k = np.asarray(k, dtype=np.float32)
    v = np.asarray(v, dtype=np.float32)
    vl = np.asarray(valid_lens).astype(np.int64)

    # valid_len == 0: reference's softmax over an all-masked row is uniform.
    # Zeroed q gives scores == 0 -> exp == const over all (unmasked) keys.
    valid_eff = np.where(vl <= 0, S, np.minimum(vl, S))
    q[vl <= 0] = 0.0

    mask = (np.arange(S)[None, :] < valid_eff[:, None]).astype(np.float32)
    qT = np.ascontiguousarray(q.transpose(0, 2, 1)).astype(np.float16)
    kT = np.ascontiguousarray(k.transpose(0, 2, 1)).astype(np.float16)
    vm = np.concatenate([v * mask[:, :, None], mask[:, :, None]], axis=2)
    vm = vm.astype(np.float16)

    # Rank-sort batches; slot s takes one batch of rank group [8s, 8s+8)
    # per core.  Schedule order: smallest first (fast fill), then largest
    # down to 2nd-smallest (cheap drain).
    order = np.argsort(-valid_eff, kind="stable")
    asc = order.reshape(NB, NCORES)[::-1]
    jts_asc = [int(np.ceil(valid_eff[asc[s]].max() / P)) for s in range(NB)]
    perm = [0] + list(range(NB - 1, 0, -1))
    assign = asc[perm]                      # [slot t, core c] -> batch
    jts = tuple(jts_asc[p] for p in perm)

    # DVE (Schraudolph) tiles: per slot, the d lowest-mass tiles -- the
    # ones fully masked for the most batches in the slot (ties: highest j).
    dve_sets = []
    for t in range(NB):
        jt = jts[t]
        d = jt - min(ACT_OF_JT[jt], jt)
        nreal = [(int(np.sum(valid_eff[assign[t]] > j * P)), -j)
                 for j in range(jt)]
        dve_sets.append(frozenset(sorted(range(jt),
                                         key=lambda j: nreal[j])[:d]))
    key = (jts, tuple(dve_sets))

    nc = _program_cache.get(key)
    if nc is None:
        nc = _build_program(key)
        _program_cache[key] = nc

    qofs, kofs, totw = _slot_layout(jts)
    in_maps = []
    for c in range(NCORES):
        blob = np.zeros((P, totw), dtype=np.float16)
        for t in range(NB):
            b = assign[t, c]
            jt = jts[t]
            qb = blob[:, qofs[t]:qofs[t] + 512]
            qb[0:D] = qT[b][:, 0:512]
            qb[D:2 * D] = qT[b][:, 512:1024]
            for j in range(jt):
                c0 = kofs[t] + j * (P + W)
                blob[0:D, c0:c0 + P] = kT[b][:, j * P:(j + 1) * P]
                blob[D:2 * D, c0:c0 + P] = kT[b][:, j * P:(j + 1) * P]
                blob[:, c0 + P:c0 + P + W] = vm[b][j * P:(j + 1) * P, :]
        in_maps.append({"inb": blob})
    cores = list(range(NCORES))
    if TRACE:
        # Warm untraced run first: NTFF profiling wrapped around the very
        # first post-compile execute has been seen to wedge the exec unit.
        try:
            bass_utils.run_bass_kernel_spmd(nc, in_maps, core_ids=cores,
                                            trace=False)
        except Exception:
            pass
    try:
        res = bass_utils.run_bass_kernel_spmd(nc, in_maps, core_ids=cores,
                                              trace=TRACE)
    except Exception:
        # One retry: transient NRT exec-unit failures have been observed on
        # the first execute after a fresh in-process NEFF compile.
        res = bass_utils.run_bass_kernel_spmd(nc, in_maps, core_ids=cores,
                                              trace=TRACE)
    LAST_RESULTS = res

    out = np.empty((B, S, D), dtype=np.float32)
    for c in range(NCORES):
        o = res.results[c]["out"]  # [NB, 128, 512] fp16
        for t in range(NB):
            out[assign[t, c]] = (
                o[t].reshape(P, 8, D).transpose(1, 0, 2)
                .reshape(S, D).astype(np.float32)
            )
    return out
